# revision 1
# baseline (speedup 1.0000x reference)
"""Trainium2 Bass kernel for CoreSageLayer (GNN mean-aggregate + 3-way linear).

Computation (reference):
    mask = (adj == 1)                      # [N, N] 0/1
    deg  = mask.sum(axis=1)                # [N]
    x1   = (mask @ x) / deg[:, None]       # [N, F]
    out[k] = concat([x1, x], 1) @ W[k] + bias   # [3, N, O]

Distribution: row-shard adj / x1 / out over nodes across 8 cores; replicate
x and weights; no collectives (rows independent).

Device schedule per core (nodes NB=1024, 8 node-tiles of 128):
  stage 1 (per node-tile j): PSUM[128, 257] accumulates 64 matmuls
      lhsT = adjT chunk [128 m, 128 n] (host pre-transposed, cast to bf16),
      rhs  = [x | ones] chunk [128 m, 257]  -> col 256 = degree (exact: 0/1
      products accumulate in fp32 PSUM).
  finalize j: rec = 1/deg (DVE), x1 = psum * rec (DVE), PE-transpose x1 into
      x1T, then stage 2: out[k, j] = [x1, x]^T-contracted matmuls in fp32.
"""

import sys

sys.path.insert(0, "/opt/trn_rl_repo")

import numpy as np

N = 8192
F = 256
O = 256
NCORES = 8
NB = N // NCORES          # nodes per core (1024)
JT = NB // 128            # node tiles per core (8)
MCHUNKS = N // 128        # contraction chunks (64)
FP = F + 1                # x columns + ones column (257)

_MASK_BF16 = True         # adjacency + x in bf16 for stage 1 (mask exact in bf16)


def _patch_tile_drain():
    """This container's walrus allows only one sync-wait per CTRL instruction;
    split the Tile kernel-tail drain's waits onto single-wait no-fuse NoOps."""
    import concourse.tile as tile
    from concourse import mybir
    from concourse.tile import ScopedClock

    if getattr(tile.TileContext, "_drain_split_patched", False):
        return

    def _drain_and_barrier(self, tick_clock, wait_clock):
        nc = self.nc
        drain_inst = nc.sync.drain()
        wait_clock.add_sem_waits(
            drain_inst.ins, ScopedClock({None: tick_clock.global_clock})
        )
        si = drain_inst.ins.sync_info
        if si is not None and len(si.on_wait) > 1:
            waits = list(si.on_wait)
            drain_inst.ins.sync_info = mybir.SyncInfo(
                on_wait=[waits[0]], on_update=list(si.on_update)
            )
            for w in waits[1:]:
                nop = nc.sync.nop(nofuse=True, hint="split_wait")
                nop.ins.sync_info = mybir.SyncInfo(on_wait=[w], on_update=[])
        nc.all_engine_barrier()
        assert self.sems is not None
        popped = nc._tile_sem_poison_stack.pop()
        assert popped is self._sem_poison
        nc.clear_and_free_semaphores(list(self.sems.allocated().values()))
        nc.all_engine_barrier()

    tile.TileContext._drain_and_barrier = _drain_and_barrier
    tile.TileContext._drain_split_patched = True

    # Same walrus limitation, general case: any instruction that Tile gave
    # >1 sem-wait (e.g. a DMA with both RAW and WAR deps) fails codegen.
    # Split surplus waits onto fresh single-wait NoOps emitted just before
    # the instruction on the same engine, at the serialized-BIR level.
    import concourse.bass as bass
    import orjson

    _orig_to_json_bytes = bass.Bass.to_json_bytes

    def _to_json_bytes_split(self):
        m = orjson.loads(_orig_to_json_bytes(self))
        ctr = 0
        for fn in m.get("functions", []):
            for bb in fn.get("blocks", []):
                insts = bb.get("instructions", [])
                new = []
                for inst in insts:
                    si = inst.get("sync_info")
                    waits = (si or {}).get("on_wait") or []
                    if len(waits) > 1:
                        for w in waits[:-1]:
                            ctr += 1
                            new.append({
                                "name": f"SWNOP-{ctr}",
                                "opcode": "NoOp",
                                "engine": inst["engine"],
                                "ins": [],
                                "outs": [],
                                "sync_info": {"on_wait": [w], "on_update": []},
                            })
                        si["on_wait"] = [waits[-1]]
                    new.append(inst)
                bb["instructions"] = new
        return orjson.dumps(m)

    bass.Bass.to_json_bytes = _to_json_bytes_split


def build_bass(with_bias: bool):
    import concourse.bass as bass
    import concourse.tile as tile
    from concourse import mybir
    from concourse.masks import make_identity

    _patch_tile_drain()

    mask_dt = mybir.dt.bfloat16 if _MASK_BF16 else mybir.dt.float32
    f32 = mybir.dt.float32

    nc = bass.Bass()
    maskt = nc.dram_tensor("maskt", [JT, 128, MCHUNKS * 128], mask_dt,
                           kind="ExternalInput")
    xp = nc.dram_tensor("xp", [128, MCHUNKS * FP], mask_dt, kind="ExternalInput")
    f32r = mybir.dt.float32r
    # stage-2 operands live as float32r (same bits as fp32 in DRAM/SBUF;
    # the PE runs fp32r matmuls at 4x the fp32 rate for free dim >= 256)
    xt = nc.dram_tensor("xt", [F // 128, 128, NB], f32r, kind="ExternalInput")
    w = nc.dram_tensor("w", [3, 2 * F // 128, 128, O], f32r, kind="ExternalInput")
    if with_bias:
        biasr = nc.dram_tensor("biasr", [128, O], f32, kind="ExternalInput")
    out = nc.dram_tensor("out", [3, NB, O], f32, kind="ExternalOutput")

    FCH = 2 * F // 128  # 4 f-chunks of 128 in the stage-2 contraction

    with tile.TileContext(nc) as tc:
        with (
            tc.tile_pool(name="const", bufs=1) as const_pool,
            tc.tile_pool(name="mask", bufs=3) as mask_pool,
            tc.tile_pool(name="work", bufs=3) as work_pool,
            tc.tile_pool(name="psum1", bufs=2, space="PSUM") as psum1_pool,
            tc.tile_pool(name="psumt", bufs=2, space="PSUM") as psumt_pool,
            tc.tile_pool(name="psum2", bufs=2, space="PSUM") as psum2_pool,
        ):
            # ---- DMA order matters: everything shares the HWDGE FIFO.
            # First the j=0 mask block (split so PE can start after the first
            # sixteenth), then x|1 (split; chunk c only needs slice c//8),
            # then weights/xT (not needed until finalize(0)).
            mt0 = mask_pool.tile([128, MCHUNKS * 128], mask_dt, tag="mt", name="mt0")
            xp_sb = const_pool.tile([128, MCHUNKS * FP], mask_dt)
            # interleave the j=0 mask quarters with the x|1 eighths so chunk c
            # (needs mask piece c//16 and xp piece c//8) unblocks at stream rate
            MSPL, XSPL = 4, 8
            mw = MCHUNKS * 128 // MSPL
            xw = MCHUNKS * FP // XSPL
            for q in range(MSPL):
                nc.sync.dma_start(mt0[:, q * mw:(q + 1) * mw],
                                  maskt[0, :, q * mw:(q + 1) * mw])
                for xq in (2 * q, 2 * q + 1):
                    nc.sync.dma_start(xp_sb[:, xq * xw:(xq + 1) * xw],
                                      xp[:, xq * xw:(xq + 1) * xw])

            def stage1(j, mt):
                ps = psum1_pool.tile([128, FP], f32, tag="acc")
                for c in range(MCHUNKS):
                    nc.tensor.matmul(
                        ps[:],
                        mt[:, c * 128:(c + 1) * 128],
                        xp_sb[:, c * FP:(c + 1) * FP],
                        start=(c == 0),
                        stop=(c == MCHUNKS - 1),
                    )
                return ps

            ps0 = stage1(0, mt0)

            # stage-2 constants: emitted after stage1(0), used by finalize(0)
            xt_sb = [const_pool.tile([128, NB], f32r, tag=f"xt{h}", name=f"xt{h}")
                     for h in range(2)]
            for h in range(2):
                nc.scalar.dma_start(xt_sb[h][:], xt[h])
            w_sb = [
                [const_pool.tile([128, O], f32r, tag=f"w{k}_{fc}", name=f"w{k}_{fc}")
                 for fc in range(FCH)]
                for k in range(3)
            ]
            for k in range(3):
                for fc in range(FCH):
                    nc.scalar.dma_start(w_sb[k][fc][:], w[k, fc])
            if with_bias:
                bias_sb = const_pool.tile([128, O], f32)
                nc.scalar.dma_start(bias_sb[:], biasr[:])
            identity = const_pool.tile([128, 128], f32)
            make_identity(nc, identity)
            # x1T[h] row f (= h*128+f), col n: x1 transposed, filled per j
            x1t_sb = [const_pool.tile([128, NB], f32r, tag=f"x1t{h}", name=f"x1t{h}")
                      for h in range(2)]

            def load_mask(j):
                mt = mask_pool.tile([128, MCHUNKS * 128], mask_dt, tag="mt",
                                    name=f"mt{j}")
                nc.sync.dma_start(mt[:], maskt[j])
                return mt

            def finalize(j, ps):
                jcols = slice(j * 128, (j + 1) * 128)
                rec = work_pool.tile([128, 1], f32, tag="rec")
                nc.vector.reciprocal(rec[:], ps[:, F:F + 1])
                x1 = work_pool.tile([128, F], f32, tag="x1")
                nc.vector.tensor_scalar_mul(x1[:], ps[:, 0:F], rec[:])
                for h in range(2):
                    pt = psumt_pool.tile([128, 128], f32, tag="pt")
                    nc.tensor.transpose(pt[:], x1[:, h * 128:(h + 1) * 128], identity[:])
                    nc.vector.tensor_copy(x1t_sb[h][:, jcols], pt[:])
                # stage-2 contraction in float32r: 4x the fp32 matmul rate at
                # near-fp32 precision (free dim 256 >= the full-rate cutoff)
                lhs = [x1t_sb[0], x1t_sb[1], xt_sb[0], xt_sb[1]]
                for k in range(3):
                    po = psum2_pool.tile([128, O], f32, tag="po")
                    for fc in range(FCH):
                        nc.tensor.matmul(
                            po[:],
                            lhs[fc][:, jcols],
                            w_sb[k][fc][:],
                            start=(fc == 0),
                            stop=(fc == FCH - 1),
                        )
                    ot = work_pool.tile([128, O], f32, tag="ot")
                    if with_bias:
                        nc.vector.tensor_add(ot[:], po[:], bias_sb[:])
                    else:
                        nc.vector.tensor_copy(ot[:], po[:])
                    nc.scalar.dma_start(out[k, j * 128:(j + 1) * 128, :], ot[:])

            # software-pipeline by one node-tile so PE never stalls on the
            # DVE reciprocal/divide between stage-1 accumulation and stage 2
            prev = (0, ps0)
            for j in range(1, JT):
                mt = load_mask(j)
                ps = stage1(j, mt)
                finalize(*prev)
                prev = (j, ps)
            finalize(*prev)

    return nc


_cached = {}


def _get_bass(with_bias: bool):
    if with_bias not in _cached:
        _cached[with_bias] = build_bass(with_bias)
    return _cached[with_bias]


def _host_prep(x, adj, weight, bias):
    import ml_dtypes

    mask_np = ml_dtypes.bfloat16 if _MASK_BF16 else np.float32
    x = np.asarray(x, dtype=np.float32)
    adj = np.asarray(adj)
    weight = np.asarray(weight, dtype=np.float32)
    bias = np.asarray(bias, dtype=np.float32)

    with_bias = bool(np.any(bias))

    # replicated: [x | 1] in stage-1 layout [128 p][chunk c][F+1]
    xp = np.empty((N, FP), dtype=np.float32)
    xp[:, :F] = x
    xp[:, F] = 1.0
    xp_t = np.ascontiguousarray(
        xp.reshape(MCHUNKS, 128, FP).transpose(1, 0, 2)
    ).reshape(128, MCHUNKS * FP).astype(mask_np)

    w_t = np.ascontiguousarray(weight.reshape(3, 2 * F // 128, 128, O))
    bias_r = np.broadcast_to(bias, (128, O)).copy() if with_bias else None

    mask = (adj == 1)
    in_maps = []
    for c in range(NCORES):
        rows = slice(c * NB, (c + 1) * NB)
        # adjT shard in DMA-friendly layout [j][p][g c n]: element
        # [j, p, g*1024 + ci*128 + n] = mask[node j*128+n, m=g*1024+ci*128+p]
        a = mask[rows].T.astype(mask_np)          # [N m, NB n]
        a = a.reshape(MCHUNKS // 8, 8, 128, JT, 128)   # [g, ci, p, j, n]
        a = np.ascontiguousarray(a.transpose(3, 2, 0, 1, 4)).reshape(
            JT, 128, MCHUNKS * 128
        )
        xt_c = np.ascontiguousarray(x[rows].T).reshape(F // 128, 128, NB)
        m = {"maskt": a, "xp": xp_t, "xt": xt_c, "w": w_t}
        if with_bias:
            m["biasr"] = bias_r
        in_maps.append(m)
    return in_maps, with_bias


def run(x, adj, weight, bias, trace=False, trace_kwargs=None):
    """Shard, run on 8 cores, gather. Returns (out_full, BassKernelResults)."""
    from concourse.bass_utils import run_bass_kernel_spmd

    in_maps, with_bias = _host_prep(x, adj, weight, bias)
    nc = _get_bass(with_bias)
    res = run_bass_kernel_spmd(
        nc, in_maps, list(range(NCORES)), trace=trace, **(trace_kwargs or {})
    )
    out_full = np.empty((3, N, O), dtype=np.float32)
    for c in range(NCORES):
        out_full[:, c * NB:(c + 1) * NB, :] = res.results[c]["out"]
    return out_full, res


def kernel(g, x, adj, weight, bias):
    out, _ = run(x, adj, weight, bias)
    return out



# revision 2
# speedup vs baseline: 1.0110x; 1.0110x over previous
"""Trainium2 Bass kernel for CoreSageLayer (GNN mean-aggregate + 3-way linear).

Computation (reference):
    mask = (adj == 1)                      # [N, N] 0/1
    deg  = mask.sum(axis=1)                # [N]
    x1   = (mask @ x) / deg[:, None]       # [N, F]
    out[k] = concat([x1, x], 1) @ W[k] + bias   # [3, N, O]

Distribution: row-shard adj / x1 / out over nodes across 8 cores; replicate
x and weights; no collectives (rows independent).

v2 (fp8): stage 1 runs in fp8e4 (mask 0/1 exact; x quantized — the mean over
~4096 neighbors averages the quantization error down to ~3e-4) using the PE's
DoubleRow perf mode: each matmul consumes TWO 128-row contraction chunks from
[128, 2, f]-sliced operands. Halves mask DMA bytes vs bf16 and >=2x stage-1
PE throughput. Stage 2 (the [3,2F,O] linear) runs in bf16 on exact fp32->bf16
inputs, which keeps the x @ W2 term's error at bf16 level (~2e-3 max).

Device schedule per core (nodes NB=1024, 8 node-tiles of 128):
  stage 1 (per node-tile j): PSUM[128, 257] accumulates 32 DoubleRow matmuls
      lhsT = adjT chunk pair [128 m, 2, 128 n] fp8,
      rhs  = [x | ones] chunk pair [128 m, 2, 257] fp8 -> col 256 = degree
      (exact: 0/1 products accumulate in fp32 PSUM).
  finalize j: rec = 1/deg (DVE), x1 = psum * rec (DVE fp32), PE-transpose x1
      into bf16 x1T, then stage 2: out[k, j] = bf16 matmuls, fp32 PSUM/out.
"""

import sys

sys.path.insert(0, "/opt/trn_rl_repo")

import numpy as np

N = 8192
F = 256
O = 256
NCORES = 8
NB = N // NCORES          # nodes per core (1024)
JT = NB // 128            # node tiles per core (8)
MCHUNKS = N // 128        # contraction chunks (64)
MPAIRS = MCHUNKS // 2     # DoubleRow chunk pairs (32)
FP = F + 1                # x columns + ones column (257)


def _patch_tile_drain():
    """This container's walrus allows only one sync-wait per CTRL instruction;
    split the Tile kernel-tail drain's waits onto single-wait no-fuse NoOps."""
    import concourse.tile as tile
    from concourse import mybir
    from concourse.tile import ScopedClock

    if getattr(tile.TileContext, "_drain_split_patched", False):
        return

    def _drain_and_barrier(self, tick_clock, wait_clock):
        nc = self.nc
        drain_inst = nc.sync.drain()
        wait_clock.add_sem_waits(
            drain_inst.ins, ScopedClock({None: tick_clock.global_clock})
        )
        si = drain_inst.ins.sync_info
        if si is not None and len(si.on_wait) > 1:
            waits = list(si.on_wait)
            drain_inst.ins.sync_info = mybir.SyncInfo(
                on_wait=[waits[0]], on_update=list(si.on_update)
            )
            for w in waits[1:]:
                nop = nc.sync.nop(nofuse=True, hint="split_wait")
                nop.ins.sync_info = mybir.SyncInfo(on_wait=[w], on_update=[])
        nc.all_engine_barrier()
        assert self.sems is not None
        popped = nc._tile_sem_poison_stack.pop()
        assert popped is self._sem_poison
        nc.clear_and_free_semaphores(list(self.sems.allocated().values()))
        nc.all_engine_barrier()

    tile.TileContext._drain_and_barrier = _drain_and_barrier
    tile.TileContext._drain_split_patched = True

    # Same walrus limitation, general case: any instruction that Tile gave
    # >1 sem-wait (e.g. a DMA with both RAW and WAR deps) fails codegen.
    # Split surplus waits onto fresh single-wait NoOps emitted just before
    # the instruction on the same engine, at the serialized-BIR level.
    import concourse.bass as bass
    import orjson

    _orig_to_json_bytes = bass.Bass.to_json_bytes

    def _to_json_bytes_split(self):
        m = orjson.loads(_orig_to_json_bytes(self))
        ctr = 0
        for fn in m.get("functions", []):
            for bb in fn.get("blocks", []):
                insts = bb.get("instructions", [])
                new = []
                for inst in insts:
                    si = inst.get("sync_info")
                    waits = (si or {}).get("on_wait") or []
                    if len(waits) > 1:
                        for w in waits[:-1]:
                            ctr += 1
                            new.append({
                                "name": f"SWNOP-{ctr}",
                                "opcode": "NoOp",
                                "engine": inst["engine"],
                                "ins": [],
                                "outs": [],
                                "sync_info": {"on_wait": [w], "on_update": []},
                            })
                        si["on_wait"] = [waits[-1]]
                    new.append(inst)
                bb["instructions"] = new
        return orjson.dumps(m)

    bass.Bass.to_json_bytes = _to_json_bytes_split


def build_bass(with_bias: bool):
    import concourse.bass as bass
    import concourse.tile as tile
    from concourse import mybir
    from concourse.masks import make_identity

    _patch_tile_drain()

    f8 = mybir.dt.float8e4
    bf16 = mybir.dt.bfloat16
    f32 = mybir.dt.float32

    nc = bass.Bass()
    maskt = nc.dram_tensor("maskt", [JT, 128, MCHUNKS * 128], f8,
                           kind="ExternalInput")
    xp = nc.dram_tensor("xp", [128, MCHUNKS * FP], f8, kind="ExternalInput")
    # all stage-2 constants in one DMA-friendly pack:
    # [128 p][xt0 (NB) | xt1 (NB) | w(k,fc) 12*O]
    CW = 2 * NB + 12 * O
    cw = nc.dram_tensor("cw", [128, CW], bf16, kind="ExternalInput")
    if with_bias:
        biasr = nc.dram_tensor("biasr", [128, O], f32, kind="ExternalInput")
    # [node n][k][O]: one 3*O-wide store per node tile; host unshuffles to
    # the reference's [3, N, O]
    out = nc.dram_tensor("out", [NB, 3 * O], f32, kind="ExternalOutput")

    FCH = 2 * F // 128  # 4 f-chunks of 128 in the stage-2 contraction
    DR = mybir.MatmulPerfMode.DoubleRow

    with tile.TileContext(nc) as tc:
        with (
            tc.tile_pool(name="const", bufs=1) as const_pool,
            tc.tile_pool(name="mask", bufs=5) as mask_pool,
            tc.tile_pool(name="work", bufs=3) as work_pool,
            tc.tile_pool(name="psum1", bufs=2, space="PSUM") as psum1_pool,
            tc.tile_pool(name="psumt", bufs=2, space="PSUM") as psumt_pool,
            tc.tile_pool(name="psum2", bufs=3, space="PSUM") as psum2_pool,
        ):
            # ---- DMA order matters: everything shares the HWDGE FIFO.
            # Interleave the j=0 mask block with x|1 in matched chunk-range
            # pieces, small first so the PE's first pair unblocks ~1us after
            # DMA data starts flowing, growing so descriptor overhead stays
            # amortized. Chunk pair c2 needs mask chunks <=2c2+1 and the same
            # xp chunks; xp piece first within each group (PE waits on both).
            mt0 = mask_pool.tile([128, MCHUNKS, 128], f8, tag="mt", name="mt0")
            xp_sb = const_pool.tile([128, MCHUNKS, FP], f8)
            PIECES = (4, 4, 8, 16, 32)
            o = 0
            for pc in PIECES:
                nc.sync.dma_start(xp_sb[:, o:o + pc, :],
                                  xp[:, o * FP:(o + pc) * FP])
                nc.sync.dma_start(mt0[:, o:o + pc, :],
                                  maskt[0, :, o * 128:(o + pc) * 128])
                o += pc
            assert o == MCHUNKS

            def stage1(j, mt, interleave=None):
                """interleave: optional list of thunks emitting PE work to
                slot between stage-1 pairs (fills stream-starved PE time)."""
                ps = psum1_pool.tile([128, FP], f32, tag="acc")
                it = list(interleave or [])
                gi = 0
                for c2 in range(MPAIRS):
                    nc.tensor.matmul(
                        ps[:],
                        mt[:, 2 * c2:2 * c2 + 2, :],
                        xp_sb[:, 2 * c2:2 * c2 + 2, :],
                        start=(c2 == 0),
                        stop=(c2 == MPAIRS - 1),
                        perf_mode=DR,
                        skip_group_check=True,
                    )
                    if gi < len(it) and c2 % 1 == 0:
                        it[gi]()
                        gi += 1
                while gi < len(it):
                    it[gi]()
                    gi += 1
                return ps

            ps0 = stage1(0, mt0)

            def load_mask(j):
                mt = mask_pool.tile([128, MCHUNKS, 128], f8, tag="mt",
                                    name=f"mt{j}")
                nc.sync.dma_start(mt[:], maskt[j])
                return mt

            # issue ALL remaining mask loads now, back-to-back on the sync
            # ring: issuing them late (interleaved with compute emission)
            # left each load semaphore-gated until ~one tile period before
            # its use, so every stage-1 start stalled on mask arrival. The
            # pool WAR sems (bufs) still pace the later loads safely.
            mts = {j: load_mask(j) for j in range(1, JT)}

            # stage-2 constants: one packed DMA on the scalar ring — runs in
            # parallel (round-robin) with the sync ring's startup stream and
            # is done well before finalize(0) needs it
            cw_sb = const_pool.tile([128, CW], bf16, name="cw")
            nc.scalar.dma_start(cw_sb[:], cw[:])
            xt_sb = [cw_sb[:, h * NB:(h + 1) * NB] for h in range(2)]
            w_sb = [
                [cw_sb[:, 2 * NB + (k * FCH + fc) * O:
                       2 * NB + (k * FCH + fc + 1) * O]
                 for fc in range(FCH)]
                for k in range(3)
            ]
            if with_bias:
                bias_sb = const_pool.tile([128, O], f32)
                nc.scalar.dma_start(bias_sb[:], biasr[:])
            identity = const_pool.tile([128, 128], f32)
            make_identity(nc, identity)
            # x1T[h] row f (= h*128+f), col n: x1 transposed, filled per j
            x1t_sb = [const_pool.tile([128, NB], bf16, tag=f"x1t{h}", name=f"x1t{h}")
                      for h in range(2)]
            # x @ W2 partials (no stage-1 dependency): precomputed for every
            # (j, k) during stage1(1)'s PE-idle slots, stashed in SBUF, and
            # added back (DVE) in finalize — drops stage-2 from 12 to 6
            # matmuls per tile on the PE-bound critical path
            p2_sb = const_pool.tile([128, 3 * JT * O], f32, name="p2")

            def pre_group(j, k):
                def emit():
                    po = psum2_pool.tile([128, O], f32, tag="po")
                    for i, fc in enumerate((2, 3)):
                        nc.tensor.matmul(
                            po[:],
                            cw_sb[:, (fc - 2) * NB + j * 128:
                                  (fc - 2) * NB + (j + 1) * 128],
                            w_sb[k][fc],
                            start=(i == 0),
                            stop=(i == 1),
                        )
                    dst = p2_sb[:, (j * 3 + k) * O:(j * 3 + k + 1) * O]
                    if with_bias:
                        nc.vector.tensor_add(dst, po[:], bias_sb[:])
                    else:
                        nc.vector.tensor_copy(dst, po[:])
                return emit

            def finalize_head(ps):
                """DVE part that frees the stage-1 psum and produces x1.
                Emitted IMMEDIATELY after stage1(j) so it sits ahead of the
                previous tile's casts/adds in the DVE's in-order queue —
                otherwise the PE's x1 transpose waits ~2.6us every tile."""
                rec = work_pool.tile([128, 1], f32, tag="rec")
                nc.vector.reciprocal(rec[:], ps[:, F:F + 1])
                x1 = work_pool.tile([128, F], f32, tag="x1")
                nc.vector.tensor_scalar_mul(x1[:], ps[:, 0:F], rec[:])
                return x1

            def finalize(j, x1, split_store=False):
                jcols = slice(j * 128, (j + 1) * 128)
                for h in range(2):
                    pt = psumt_pool.tile([128, 128], f32, tag="pt")
                    nc.tensor.transpose(pt[:], x1[:, h * 128:(h + 1) * 128], identity[:])
                    nc.vector.tensor_copy(x1t_sb[h][:, jcols], pt[:])
                lhs = [x1t_sb[0][:, jcols], x1t_sb[1][:, jcols]]
                ot = work_pool.tile([128, 3 * O], f32, tag="ot")
                for k in range(3):
                    po = psum2_pool.tile([128, O], f32, tag="po")
                    for fc in range(2):
                        nc.tensor.matmul(
                            po[:],
                            lhs[fc],
                            w_sb[k][fc],
                            start=(fc == 0),
                            stop=(fc == 1),
                        )
                    # out = x1@W1 (psum) + stashed x@W2 (+bias, pre-folded)
                    nc.vector.tensor_add(
                        ot[:, k * O:(k + 1) * O], po[:],
                        p2_sb[:, (j * 3 + k) * O:(j * 3 + k + 1) * O])
                    if split_store:
                        # last tile: store per k so the store stream overlaps
                        # the remaining stage-2 matmuls instead of trailing
                        # (HWDGE: lowest latency; all loads are done by now)
                        nc.scalar.dma_start(
                            out[j * 128:(j + 1) * 128, k * O:(k + 1) * O],
                            ot[:, k * O:(k + 1) * O])
                if not split_store:
                    # SWDGE path: keeps the shared 8 HWDGE semaphore lanes
                    # loads-only — a store on a lane otherwise serializes a
                    # later mask load behind this tile's compute
                    nc.gpsimd.dma_start(out[j * 128:(j + 1) * 128, :], ot[:])

            # software-pipeline by one node-tile so PE never stalls on the
            # DVE reciprocal/divide between stage-1 accumulation and stage 2
            prev = (0, finalize_head(ps0))
            pre = [pre_group(j, k) for j in range(JT) for k in range(3)]
            for j in range(1, JT):
                ps = stage1(j, mts[j], interleave=pre if j == 1 else None)
                head = finalize_head(ps)
                finalize(prev[0], prev[1])
                prev = (j, head)
            finalize(prev[0], prev[1], split_store=True)

    return nc


_cached = {}


def _get_bass(with_bias: bool):
    if with_bias not in _cached:
        _cached[with_bias] = build_bass(with_bias)
    return _cached[with_bias]


def _host_prep(x, adj, weight, bias):
    import ml_dtypes

    f8 = ml_dtypes.float8_e4m3
    bf = ml_dtypes.bfloat16
    x = np.asarray(x, dtype=np.float32)
    adj = np.asarray(adj)
    weight = np.asarray(weight, dtype=np.float32)
    bias = np.asarray(bias, dtype=np.float32)

    with_bias = bool(np.any(bias))

    # replicated: [x | 1] in stage-1 layout [128 p][chunk c][F+1], fp8
    xp = np.empty((N, FP), dtype=np.float32)
    xp[:, :F] = x
    xp[:, F] = 1.0
    xp_t = np.ascontiguousarray(
        xp.reshape(MCHUNKS, 128, FP).transpose(1, 0, 2)
    ).reshape(128, MCHUNKS * FP).astype(f8)

    # packed stage-2 weights part: [128 p][3 k][4 fc][O] (per-core xt is
    # prepended in the loop below)
    w_t = np.ascontiguousarray(
        weight.reshape(3, 2 * F // 128, 128, O).transpose(2, 0, 1, 3)
    ).reshape(128, 12 * O).astype(bf)
    bias_r = np.broadcast_to(bias, (128, O)).copy() if with_bias else None

    mask_u8 = (adj == 1).view(np.uint8) if adj.dtype == np.bool_ else (
        (adj == 1).astype(np.uint8))
    in_maps = []
    for c in range(NCORES):
        rows = slice(c * NB, (c + 1) * NB)
        # adjT shard in DMA-friendly layout [j][p][c n]: element
        # [j, p, ci*128 + n] = mask[node j*128+n, m=ci*128+p], as fp8
        # (0/1 -> bytes 0x00/0x38 == e4m3 0.0/1.0; avoids slow generic cast)
        a = mask_u8[rows].T                        # [N m, NB n] u8 0/1
        a = a.reshape(MCHUNKS, 128, JT, 128)       # [ci, p, j, n]
        a = (np.ascontiguousarray(a.transpose(2, 1, 0, 3)) * np.uint8(0x38)
             ).reshape(JT, 128, MCHUNKS * 128).view(f8)
        # cw pack: [128 p][xt0 NB | xt1 NB | w 12*O], xt[h] row p col n =
        # x[rows][n, h*128+p]
        xt_c = x[rows].T.reshape(2, 128, NB)
        cw_c = np.empty((128, 2 * NB + 12 * O), dtype=bf)
        cw_c[:, :NB] = xt_c[0]
        cw_c[:, NB:2 * NB] = xt_c[1]
        cw_c[:, 2 * NB:] = w_t
        m = {"maskt": a, "xp": xp_t, "cw": cw_c}
        if with_bias:
            m["biasr"] = bias_r
        in_maps.append(m)
    return in_maps, with_bias


def run(x, adj, weight, bias, trace=False, trace_kwargs=None):
    """Shard, run on 8 cores, gather. Returns (out_full, BassKernelResults)."""
    from concourse.bass_utils import run_bass_kernel_spmd

    in_maps, with_bias = _host_prep(x, adj, weight, bias)
    nc = _get_bass(with_bias)
    res = run_bass_kernel_spmd(
        nc, in_maps, list(range(NCORES)), trace=trace, **(trace_kwargs or {})
    )
    out_full = np.empty((3, N, O), dtype=np.float32)
    for c in range(NCORES):
        out_full[:, c * NB:(c + 1) * NB, :] = (
            res.results[c]["out"].reshape(NB, 3, O).transpose(1, 0, 2))
    return out_full, res


def kernel(g, x, adj, weight, bias):
    out, _ = run(x, adj, weight, bias)
    return out


# revision 3
# speedup vs baseline: 1.0208x; 1.0097x over previous
"""Trainium2 Bass kernel for CoreSageLayer (GNN mean-aggregate + 3-way linear).

Computation (reference):
    mask = (adj == 1)                      # [N, N] 0/1
    deg  = mask.sum(axis=1)                # [N]
    x1   = (mask @ x) / deg[:, None]       # [N, F]
    out[k] = concat([x1, x], 1) @ W[k] + bias   # [3, N, O]

Distribution: row-shard adj / x1 / out over nodes across 8 cores; replicate
x and weights; no collectives (rows independent).

v2 (fp8): stage 1 runs in fp8e4 (mask 0/1 exact; x quantized — the mean over
~4096 neighbors averages the quantization error down to ~3e-4) using the PE's
DoubleRow perf mode: each matmul consumes TWO 128-row contraction chunks from
[128, 2, f]-sliced operands. Halves mask DMA bytes vs bf16 and >=2x stage-1
PE throughput. Stage 2 (the [3,2F,O] linear) runs in bf16 on exact fp32->bf16
inputs, which keeps the x @ W2 term's error at bf16 level (~2e-3 max).

Device schedule per core (nodes NB=1024, 8 node-tiles of 128):
  stage 1 (per node-tile j): PSUM[128, 257] accumulates 32 DoubleRow matmuls
      lhsT = adjT chunk pair [128 m, 2, 128 n] fp8,
      rhs  = [x | ones] chunk pair [128 m, 2, 257] fp8 -> col 256 = degree
      (exact: 0/1 products accumulate in fp32 PSUM).
  finalize j: rec = 1/deg (DVE), x1 = psum * rec (DVE fp32), PE-transpose x1
      into bf16 x1T, then stage 2: out[k, j] = bf16 matmuls, fp32 PSUM/out.
"""

import sys

sys.path.insert(0, "/opt/trn_rl_repo")

import numpy as np

N = 8192
F = 256
O = 256
NCORES = 8
NB = N // NCORES          # nodes per core (1024)
JT = NB // 128            # node tiles per core (8)
MCHUNKS = N // 128        # contraction chunks (64)
MPAIRS = MCHUNKS // 2     # DoubleRow chunk pairs (32)
FP = F + 1                # x columns + ones column (257)


def _patch_tile_drain():
    """This container's walrus allows only one sync-wait per CTRL instruction;
    split the Tile kernel-tail drain's waits onto single-wait no-fuse NoOps."""
    import concourse.tile as tile
    from concourse import mybir
    from concourse.tile import ScopedClock

    if getattr(tile.TileContext, "_drain_split_patched", False):
        return

    def _drain_and_barrier(self, tick_clock, wait_clock):
        nc = self.nc
        drain_inst = nc.sync.drain()
        wait_clock.add_sem_waits(
            drain_inst.ins, ScopedClock({None: tick_clock.global_clock})
        )
        si = drain_inst.ins.sync_info
        if si is not None and len(si.on_wait) > 1:
            waits = list(si.on_wait)
            drain_inst.ins.sync_info = mybir.SyncInfo(
                on_wait=[waits[0]], on_update=list(si.on_update)
            )
            for w in waits[1:]:
                nop = nc.sync.nop(nofuse=True, hint="split_wait")
                nop.ins.sync_info = mybir.SyncInfo(on_wait=[w], on_update=[])
        nc.all_engine_barrier()
        assert self.sems is not None
        popped = nc._tile_sem_poison_stack.pop()
        assert popped is self._sem_poison
        nc.clear_and_free_semaphores(list(self.sems.allocated().values()))
        nc.all_engine_barrier()

    tile.TileContext._drain_and_barrier = _drain_and_barrier
    tile.TileContext._drain_split_patched = True

    # Same walrus limitation, general case: any instruction that Tile gave
    # >1 sem-wait (e.g. a DMA with both RAW and WAR deps) fails codegen.
    # Split surplus waits onto fresh single-wait NoOps emitted just before
    # the instruction on the same engine, at the serialized-BIR level.
    import concourse.bass as bass
    import orjson

    _orig_to_json_bytes = bass.Bass.to_json_bytes

    def _to_json_bytes_split(self):
        m = orjson.loads(_orig_to_json_bytes(self))
        ctr = 0
        for fn in m.get("functions", []):
            for bb in fn.get("blocks", []):
                insts = bb.get("instructions", [])
                new = []
                for inst in insts:
                    si = inst.get("sync_info")
                    waits = (si or {}).get("on_wait") or []
                    if len(waits) > 1:
                        for w in waits[:-1]:
                            ctr += 1
                            new.append({
                                "name": f"SWNOP-{ctr}",
                                "opcode": "NoOp",
                                "engine": inst["engine"],
                                "ins": [],
                                "outs": [],
                                "sync_info": {"on_wait": [w], "on_update": []},
                            })
                        si["on_wait"] = [waits[-1]]
                    new.append(inst)
                bb["instructions"] = new
        return orjson.dumps(m)

    bass.Bass.to_json_bytes = _to_json_bytes_split


def build_bass(with_bias: bool):
    import concourse.bass as bass
    import concourse.tile as tile
    from concourse import mybir
    from concourse.masks import make_identity

    _patch_tile_drain()

    f8 = mybir.dt.float8e4
    bf16 = mybir.dt.bfloat16
    f32 = mybir.dt.float32

    nc = bass.Bass()
    maskt = nc.dram_tensor("maskt", [JT, 128, MCHUNKS * 128], f8,
                           kind="ExternalInput")
    xp = nc.dram_tensor("xp", [128, MCHUNKS * FP], f8, kind="ExternalInput")
    # all stage-2 constants in one DMA-friendly pack:
    # [128 p][xt0 (NB) | xt1 (NB) | w(k,fc) 12*O]
    CW = 2 * NB + 12 * O
    cw = nc.dram_tensor("cw", [128, CW], bf16, kind="ExternalInput")
    if with_bias:
        biasr = nc.dram_tensor("biasr", [128, O], f32, kind="ExternalInput")
    # [node n][k][O]: one 3*O-wide store per node tile; host unshuffles to
    # the reference's [3, N, O]
    out = nc.dram_tensor("out", [NB, 3 * O], f32, kind="ExternalOutput")

    FCH = 2 * F // 128  # 4 f-chunks of 128 in the stage-2 contraction
    DR = mybir.MatmulPerfMode.DoubleRow

    with tile.TileContext(nc) as tc:
        with (
            tc.tile_pool(name="const", bufs=1) as const_pool,
            tc.tile_pool(name="mask", bufs=5) as mask_pool,
            tc.tile_pool(name="work", bufs=3) as work_pool,
            tc.tile_pool(name="psum1", bufs=2, space="PSUM") as psum1_pool,
            tc.tile_pool(name="psumt", bufs=2, space="PSUM") as psumt_pool,
            tc.tile_pool(name="psum2", bufs=3, space="PSUM") as psum2_pool,
            tc.tile_pool(name="psumw", bufs=1, space="PSUM") as psumw_pool,
        ):
            # ---- DMA order matters: everything shares the HWDGE FIFO.
            # Interleave the j=0 mask block with x|1 in matched chunk-range
            # pieces, small first so the PE's first pair unblocks ~1us after
            # DMA data starts flowing, growing so descriptor overhead stays
            # amortized. Chunk pair c2 needs mask chunks <=2c2+1 and the same
            # xp chunks; xp piece first within each group (PE waits on both).
            mt0 = mask_pool.tile([128, MCHUNKS, 128], f8, tag="mt", name="mt0")
            xp_sb = const_pool.tile([128, MCHUNKS, FP], f8)

            # PE warm-up: the tensor engine runs ~1.5x slower until ~3us of
            # continuous execution (DVFS ramp). Burn dummy identity matmuls
            # into a scratch PSUM bank during the otherwise-idle window while
            # the first DMA pieces stream, so real matmuls start at full clock.
            identity = const_pool.tile([128, 128], f32)
            make_identity(nc, identity)
            scr = psumw_pool.tile([128, 128], f32)
            for _ in range(38):
                nc.tensor.matmul(scr[:], identity[:], identity[:],
                                 start=True, stop=True)

            PIECES = (4, 4, 8, 16, 32)
            o = 0
            for pc in PIECES:
                nc.sync.dma_start(xp_sb[:, o:o + pc, :],
                                  xp[:, o * FP:(o + pc) * FP])
                nc.sync.dma_start(mt0[:, o:o + pc, :],
                                  maskt[0, :, o * 128:(o + pc) * 128])
                o += pc
            assert o == MCHUNKS

            def stage1(j, mt, interleave=None):
                """interleave: optional list of thunks emitting PE work to
                slot between stage-1 pairs (fills stream-starved PE time)."""
                ps = psum1_pool.tile([128, FP], f32, tag="acc")
                it = list(interleave or [])
                gi = 0
                for c2 in range(MPAIRS):
                    nc.tensor.matmul(
                        ps[:],
                        mt[:, 2 * c2:2 * c2 + 2, :],
                        xp_sb[:, 2 * c2:2 * c2 + 2, :],
                        start=(c2 == 0),
                        stop=(c2 == MPAIRS - 1),
                        perf_mode=DR,
                        skip_group_check=True,
                    )
                    if gi < len(it) and c2 % 1 == 0:
                        it[gi]()
                        gi += 1
                while gi < len(it):
                    it[gi]()
                    gi += 1
                return ps

            ps0 = stage1(0, mt0)

            def load_mask(j):
                mt = mask_pool.tile([128, MCHUNKS, 128], f8, tag="mt",
                                    name=f"mt{j}")
                nc.sync.dma_start(mt[:], maskt[j])
                return mt

            # issue ALL remaining mask loads now, back-to-back on the sync
            # ring: issuing them late (interleaved with compute emission)
            # left each load semaphore-gated until ~one tile period before
            # its use, so every stage-1 start stalled on mask arrival. The
            # pool WAR sems (bufs) still pace the later loads safely.
            mts = {j: load_mask(j) for j in range(1, JT)}

            # stage-2 constants: one packed DMA on the scalar ring — runs in
            # parallel (round-robin) with the sync ring's startup stream and
            # is done well before finalize(0) needs it
            cw_sb = const_pool.tile([128, CW], bf16, name="cw")
            nc.scalar.dma_start(cw_sb[:], cw[:])
            xt_sb = [cw_sb[:, h * NB:(h + 1) * NB] for h in range(2)]
            w_sb = [
                [cw_sb[:, 2 * NB + (k * FCH + fc) * O:
                       2 * NB + (k * FCH + fc + 1) * O]
                 for fc in range(FCH)]
                for k in range(3)
            ]
            if with_bias:
                bias_sb = const_pool.tile([128, O], f32)
                nc.scalar.dma_start(bias_sb[:], biasr[:])
            # x1T[h] row f (= h*128+f), col n: x1 transposed, filled per j
            x1t_sb = [const_pool.tile([128, NB], bf16, tag=f"x1t{h}", name=f"x1t{h}")
                      for h in range(2)]
            # x @ W2 partials (no stage-1 dependency): precomputed for every
            # (j, k) during stage1(1)'s PE-idle slots, stashed in SBUF, and
            # added back (DVE) in finalize — drops stage-2 from 12 to 6
            # matmuls per tile on the PE-bound critical path
            p2_sb = const_pool.tile([128, 3 * JT * O], f32, name="p2")

            def pre_group(j, k):
                def emit():
                    po = psum2_pool.tile([128, O], f32, tag="po")
                    for i, fc in enumerate((2, 3)):
                        nc.tensor.matmul(
                            po[:],
                            cw_sb[:, (fc - 2) * NB + j * 128:
                                  (fc - 2) * NB + (j + 1) * 128],
                            w_sb[k][fc],
                            start=(i == 0),
                            stop=(i == 1),
                        )
                    dst = p2_sb[:, (j * 3 + k) * O:(j * 3 + k + 1) * O]
                    if with_bias:
                        nc.vector.tensor_add(dst, po[:], bias_sb[:])
                    else:
                        nc.vector.tensor_copy(dst, po[:])
                return emit

            def finalize_head(ps):
                """DVE part that frees the stage-1 psum and produces x1.
                Emitted IMMEDIATELY after stage1(j) so it sits ahead of the
                previous tile's casts/adds in the DVE's in-order queue —
                otherwise the PE's x1 transpose waits ~2.6us every tile."""
                rec = work_pool.tile([128, 1], f32, tag="rec")
                nc.vector.reciprocal(rec[:], ps[:, F:F + 1])
                x1 = work_pool.tile([128, F], f32, tag="x1")
                nc.vector.tensor_scalar_mul(x1[:], ps[:, 0:F], rec[:])
                return x1

            def finalize(j, x1, split_store=False):
                jcols = slice(j * 128, (j + 1) * 128)
                for h in range(2):
                    pt = psumt_pool.tile([128, 128], f32, tag="pt")
                    nc.tensor.transpose(pt[:], x1[:, h * 128:(h + 1) * 128], identity[:])
                    nc.vector.tensor_copy(x1t_sb[h][:, jcols], pt[:])
                lhs = [x1t_sb[0][:, jcols], x1t_sb[1][:, jcols]]
                ot = work_pool.tile([128, 3 * O], f32, tag="ot")
                for k in range(3):
                    po = psum2_pool.tile([128, O], f32, tag="po")
                    for fc in range(2):
                        nc.tensor.matmul(
                            po[:],
                            lhs[fc],
                            w_sb[k][fc],
                            start=(fc == 0),
                            stop=(fc == 1),
                        )
                    # out = x1@W1 (psum) + stashed x@W2 (+bias, pre-folded)
                    nc.vector.tensor_add(
                        ot[:, k * O:(k + 1) * O], po[:],
                        p2_sb[:, (j * 3 + k) * O:(j * 3 + k + 1) * O])
                    if split_store:
                        # last tile: store per k so the store stream overlaps
                        # the remaining stage-2 matmuls instead of trailing
                        # (HWDGE: lowest latency; all loads are done by now)
                        nc.scalar.dma_start(
                            out[j * 128:(j + 1) * 128, k * O:(k + 1) * O],
                            ot[:, k * O:(k + 1) * O])
                if not split_store:
                    # SWDGE path: keeps the shared 8 HWDGE semaphore lanes
                    # loads-only — a store on a lane otherwise serializes a
                    # later mask load behind this tile's compute
                    nc.gpsimd.dma_start(out[j * 128:(j + 1) * 128, :], ot[:])

            # software-pipeline by one node-tile so PE never stalls on the
            # DVE reciprocal/divide between stage-1 accumulation and stage 2
            prev = (0, finalize_head(ps0))
            pre = [pre_group(j, k) for j in range(JT) for k in range(3)]
            for j in range(1, JT):
                ps = stage1(j, mts[j], interleave=pre if j == 1 else None)
                head = finalize_head(ps)
                finalize(prev[0], prev[1])
                prev = (j, head)
            finalize(prev[0], prev[1], split_store=True)

    return nc


_cached = {}


def _get_bass(with_bias: bool):
    if with_bias not in _cached:
        _cached[with_bias] = build_bass(with_bias)
    return _cached[with_bias]


def _host_prep(x, adj, weight, bias):
    import ml_dtypes

    f8 = ml_dtypes.float8_e4m3
    bf = ml_dtypes.bfloat16
    x = np.asarray(x, dtype=np.float32)
    adj = np.asarray(adj)
    weight = np.asarray(weight, dtype=np.float32)
    bias = np.asarray(bias, dtype=np.float32)

    with_bias = bool(np.any(bias))

    # replicated: [x | 1] in stage-1 layout [128 p][chunk c][F+1], fp8
    xp = np.empty((N, FP), dtype=np.float32)
    xp[:, :F] = x
    xp[:, F] = 1.0
    xp_t = np.ascontiguousarray(
        xp.reshape(MCHUNKS, 128, FP).transpose(1, 0, 2)
    ).reshape(128, MCHUNKS * FP).astype(f8)

    # packed stage-2 weights part: [128 p][3 k][4 fc][O] (per-core xt is
    # prepended in the loop below)
    w_t = np.ascontiguousarray(
        weight.reshape(3, 2 * F // 128, 128, O).transpose(2, 0, 1, 3)
    ).reshape(128, 12 * O).astype(bf)
    bias_r = np.broadcast_to(bias, (128, O)).copy() if with_bias else None

    mask_u8 = (adj == 1).view(np.uint8) if adj.dtype == np.bool_ else (
        (adj == 1).astype(np.uint8))
    in_maps = []
    for c in range(NCORES):
        rows = slice(c * NB, (c + 1) * NB)
        # adjT shard in DMA-friendly layout [j][p][c n]: element
        # [j, p, ci*128 + n] = mask[node j*128+n, m=ci*128+p], as fp8
        # (0/1 -> bytes 0x00/0x38 == e4m3 0.0/1.0; avoids slow generic cast)
        a = mask_u8[rows].T                        # [N m, NB n] u8 0/1
        a = a.reshape(MCHUNKS, 128, JT, 128)       # [ci, p, j, n]
        a = (np.ascontiguousarray(a.transpose(2, 1, 0, 3)) * np.uint8(0x38)
             ).reshape(JT, 128, MCHUNKS * 128).view(f8)
        # cw pack: [128 p][xt0 NB | xt1 NB | w 12*O], xt[h] row p col n =
        # x[rows][n, h*128+p]
        xt_c = x[rows].T.reshape(2, 128, NB)
        cw_c = np.empty((128, 2 * NB + 12 * O), dtype=bf)
        cw_c[:, :NB] = xt_c[0]
        cw_c[:, NB:2 * NB] = xt_c[1]
        cw_c[:, 2 * NB:] = w_t
        m = {"maskt": a, "xp": xp_t, "cw": cw_c}
        if with_bias:
            m["biasr"] = bias_r
        in_maps.append(m)
    return in_maps, with_bias


def run(x, adj, weight, bias, trace=False, trace_kwargs=None):
    """Shard, run on 8 cores, gather. Returns (out_full, BassKernelResults)."""
    from concourse.bass_utils import run_bass_kernel_spmd

    in_maps, with_bias = _host_prep(x, adj, weight, bias)
    nc = _get_bass(with_bias)
    res = run_bass_kernel_spmd(
        nc, in_maps, list(range(NCORES)), trace=trace, **(trace_kwargs or {})
    )
    out_full = np.empty((3, N, O), dtype=np.float32)
    for c in range(NCORES):
        out_full[:, c * NB:(c + 1) * NB, :] = (
            res.results[c]["out"].reshape(NB, 3, O).transpose(1, 0, 2))
    return out_full, res


def kernel(g, x, adj, weight, bias):
    out, _ = run(x, adj, weight, bias)
    return out
